# revision 20
# baseline (speedup 1.0000x reference)
"""ODE-RNN decoder kernel for Trainium2 (8 NeuronCores, data-parallel).

Math per scan step (t = 0..98), per trajectory:
    y_ode = y + (tanh(y @ Wo1 + bo1) @ Wo2 + bo2) * dt_t
    z     = sigmoid(tanh([y_ode;x] @ Wz1 + bz1) @ Wz2 + bz2)
    r     = sigmoid(tanh([y_ode;x] @ Wr1 + br1) @ Wr2 + br2)
    h     = tanh(tanh([r*y_ode;x] @ Wh1 + bh1) @ Wh2 + bh2)
    y     = (1-z)*h + z*y_ode

Layout: feature-major on-chip ([feature, batch]); batch 8192 sharded 8 ways
data-parallel (1024/core, weights replicated), CH=2 chunks of 512 columns.

The step is latency-bound by the recurrence's serial chain, so the kernel
minimizes that cycle rather than engine throughput:

- State is kept SPLIT as y_t = h_t + p_t - s_t with h = gate output,
  p = z*y_ode, s = z*h.  y is never materialized in the loop; GEMMs contract
  h, p, s as separate accumulating matmuls (negated weights for the s part).
  p is computable mid-step; only s = z*h needs the just-produced h, so the
  post-tanh tail is a single elementwise multiply.
- The ODE update is folded into the gate layer-1 GEMMs algebraically:
      Wz1y^T @ y_ode = Wz1f^T@[h;x;1] + Wz1y^T@p - Wz1y^T@s
                       + (dt*[Wo2;bo2]@Wz1y)^T@[tode;1]
  so the gate path needs only tode (the ODE tanh), never y_ode.  The exact
  y_ode is still computed on the side (m = p-s in the step head;
  q = m + dt*p2; y_ode = h + q) for the elementwise r*y_ode / z*y_ode, but
  those sit OFF the critical cycle.
- Every bias rides a ones-row in a moving operand; activations carry no bias
  so the z/r layer-1 tanh fuses into ONE instruction over a 2-bank PSUM tile.
- z/r layer-2 stack on partitions 0:64 (r) / 64:128 (z) of one PSUM bank so
  ONE sigmoid covers both; f32r matmuls cannot write PSUM base 64, so both
  layer-2 weights and their moving tanh tile are bf16.
- Tensor-op INPUTS must share a partition base (outputs are free), so z is
  copied once per step from sigmoid rows 64:128 to a base-0 tile (off-cycle,
  Pool engine); everything else stays at base 0.

Critical cycle per step: a5(h) -> s-mul -> m1c -> a1(tode) -> ez-matmul ->
a2 -> l2 -> a3(sigmoid) -> v2(r*yode) -> m7 -> a4 -> m8 -> a5.
"""

import os
import sys

sys.path.insert(0, "/opt/trn_rl_repo")

from contextlib import ExitStack

import numpy as np

import concourse.bass as bass
import concourse.tile as tile
from concourse import bacc, mybir
from concourse.bass_utils import run_bass_kernel_spmd

N_TRAJ, T, DD, DL, NU = 8192, 100, 32, 64, 100
NSTEP = T - 1
NCORES = 8
B = N_TRAJ // NCORES  # 1024 per core
CH = int(os.environ.get("KCH", "2"))    # chunks in flight per core
NCH = B // CH                           # columns per chunk
assert NCH % 8 == 0 and NCH >= 256      # f32r matmul free-dim rules

F32 = mybir.dt.float32
F32R = mybir.dt.float32r
BF16 = mybir.dt.bfloat16
TANH = mybir.ActivationFunctionType.Tanh
SIG = mybir.ActivationFunctionType.Sigmoid
ADD = mybir.AluOpType.add
MULT = mybir.AluOpType.mult


def _build():
    nc = bacc.Bacc("TRN2", target_bir_lowering=False, debug=False)

    def din(name, shape, dt=F32R):
        return nc.dram_tensor(name, list(shape), dt, kind="ExternalInput")

    K1 = DL + DD + 1  # 97: [h; x; 1]

    xs = din("xs", [NSTEP, DD + 1, B])     # host: data[:,1:,:].T + ones row
    prior = din("prior", [DL, B])
    wo1f = din("wo1f", [K1, NU])           # [Wo1; 0; bo1]
    wo1y = din("wo1y", [DL, NU])           # Wo1
    wo1n = din("wo1n", [DL, NU])           # -Wo1
    wo2b = din("wo2b", [NU + 1, DL])       # [Wo2; bo2]
    wz1f = din("wz1f", [K1, NU])           # [Wz1; bz1]
    wz1y = din("wz1y", [DL, NU])           # Wz1[:64]
    wz1n = din("wz1n", [DL, NU])           # -Wz1[:64]
    ezb0 = din("ezb0", [NU + 1, NU])       # dt0 * [Wo2;bo2] @ Wz1y
    ezb = din("ezb", [NU + 1, NU])         # dtr * [Wo2;bo2] @ Wz1y
    wr1f = din("wr1f", [K1, NU])
    wr1y = din("wr1y", [DL, NU])
    wr1n = din("wr1n", [DL, NU])
    erb0 = din("erb0", [NU + 1, NU])
    erb = din("erb", [NU + 1, NU])
    wh1f = din("wh1f", [K1, NU])
    wz2b = din("wz2b", [NU + 1, DL], BF16)  # [Wz2; bz2]
    wr2b = din("wr2b", [NU + 1, DL], BF16)  # [Wr2; br2]
    wh2b = din("wh2b", [NU + 1, DL])        # [Wh2; bh2]
    dts = din("dts", [DL, NSTEP], F32)      # exact per-step dt (q path)
    zeros = din("zeros", [DL, B])           # p_0 = s_0 = m_0 = 0
    ones = din("ones", [1, B])              # f32r ones rows
    ones16 = din("ones16", [1, 2 * B], BF16)
    yout = nc.dram_tensor("yout", [DL, B], F32R, kind="ExternalOutput")

    mmul = nc.tensor.matmul

    with tile.TileContext(nc) as tc, ExitStack() as ctx:
        singles = ctx.enter_context(tc.tile_pool(name="singles", bufs=1))
        psum = ctx.enter_context(tc.tile_pool(name="psum", bufs=1, space="PSUM"))

        def load(dr, shape, dt=F32R):
            t_ = singles.tile(shape, dt, tag=dr.name, name="s_" + dr.name)
            nc.sync.dma_start(t_[:], dr.ap())
            return t_

        s_wo1f = load(wo1f, [K1, NU])
        s_wo1y = load(wo1y, [DL, NU])
        s_wo1n = load(wo1n, [DL, NU])
        s_wo2b = load(wo2b, [NU + 1, DL])
        s_wz1f = load(wz1f, [K1, NU])
        s_wz1y = load(wz1y, [DL, NU])
        s_wz1n = load(wz1n, [DL, NU])
        s_ezb0 = load(ezb0, [NU + 1, NU])
        s_ezb = load(ezb, [NU + 1, NU])
        s_wr1f = load(wr1f, [K1, NU])
        s_wr1y = load(wr1y, [DL, NU])
        s_wr1n = load(wr1n, [DL, NU])
        s_erb0 = load(erb0, [NU + 1, NU])
        s_erb = load(erb, [NU + 1, NU])
        s_wh1f = load(wh1f, [K1, NU])
        s_wz2b = load(wz2b, [NU + 1, DL], BF16)
        s_wr2b = load(wr2b, [NU + 1, DL], BF16)
        s_wh2b = load(wh2b, [NU + 1, DL])
        s_dts = load(dts, [DL, NSTEP], F32)

        # per-chunk persistent state tiles
        st = {}
        for c in range(CH):
            cs = slice(c * NCH, (c + 1) * NCH)
            # double-buffered [h; x; 1]: step t reads hx[t%2]; a5/DMA of step
            # t write h_{t+1}/x_{t+1} into hx[(t+1)%2]
            hx = []
            for j in range(2):
                hxj = singles.tile([K1, NCH], F32R, tag=f"hx{c}_{j}",
                                   name=f"hx{c}_{j}")
                hx.append(hxj)
            nc.sync.dma_start(hx[0][0:DL, :], prior.ap()[:, cs])
            nc.sync.dma_start(hx[0][DL:K1, :], xs.ap()[0, :, cs])
            rx = singles.tile([K1, NCH], F32R, tag=f"rx{c}", name=f"rx{c}")
            p = singles.tile([DL, NCH], F32R, tag=f"p{c}", name=f"p{c}")
            nc.sync.dma_start(p[:], zeros.ap()[:, cs])
            sg = singles.tile([DL, NCH], F32R, tag=f"sg{c}", name=f"sg{c}")
            nc.sync.dma_start(sg[:], zeros.ap()[:, cs])
            m = singles.tile([DL, NCH], F32, tag=f"m{c}", name=f"m{c}")
            tode = singles.tile([NU + 1, NCH], F32R, tag=f"to{c}", name=f"to{c}")
            nc.sync.dma_start(tode[NU:NU + 1, :], ones.ap()[:, 0:NCH])
            tzr = singles.tile([NU + 1, 2 * NCH], BF16, tag=f"tzr{c}",
                               name=f"tzr{c}")
            nc.sync.dma_start(tzr[NU:NU + 1, :], ones16.ap()[:, 0:2 * NCH])
            th = singles.tile([NU + 1, NCH], F32R, tag=f"th{c}", name=f"th{c}")
            nc.sync.dma_start(th[NU:NU + 1, :], ones.ap()[:, 0:NCH])
            st[c] = dict(
                hx=hx, rx=rx, p=p, sg=sg, m=m, tode=tode, tzr=tzr, th=th,
                q=singles.tile([DL, NCH], F32, tag=f"q{c}", name=f"q{c}"),
                yode=singles.tile([DL, NCH], F32, tag=f"yo{c}", name=f"yo{c}"),
                zc=singles.tile([DL, NCH], F32, tag=f"zc{c}", name=f"zc{c}"),
                szr=singles.tile([2 * DL, NCH], F32, tag=f"szr{c}",
                                 name=f"szr{c}"),
            )

        # Stage-major emission across chunks: each engine's in-order queue
        # alternates chunks per stage, so when chunk A's next op waits on a
        # recurrence dependency, chunk B's same-stage op behind it is already
        # satisfied (no head-of-line blocking).
        chunks = list(range(CH))
        for t in range(NSTEP):
            p1 = {}; p2 = {}; pzr1 = {}; pzr2 = {}; ph = {}; ph2 = {}
            ez = s_ezb0 if t == 0 else s_ezb
            er = s_erb0 if t == 0 else s_erb
            for c in chunks:
                s = st[c]
                cs = slice(c * NCH, (c + 1) * NCH)
                nxt = s["hx"][(t + 1) % 2]
                if t + 1 < NSTEP:
                    nc.sync.dma_start(nxt[DL:K1, :], xs.ap()[t + 1, :, cs])
                nc.sync.dma_start(s["rx"][DL:K1, :], xs.ap()[t, :, cs])
            # m = p - s in the idle step head (feeds the exact y_ode path)
            for c in chunks:
                s = st[c]
                nc.gpsimd.tensor_sub(s["m"][:], s["p"][:].bitcast(F32),
                                     s["sg"][:].bitcast(F32))
            # ODE l1: p1 = Wo1^T(h+p-s) + bo1 (s-part last: s lands latest)
            for c in chunks:
                s = st[c]
                cur = s["hx"][t % 2]
                p1[c] = psum.tile([NU, NCH], F32, tag=f"l1{c}", name="p1",
                                  bufs=1)
                mmul(p1[c][:], s_wo1f[:], cur[:], start=True, stop=False)
                mmul(p1[c][:], s_wo1y[:], s["p"][:], start=False, stop=False)
                mmul(p1[c][:], s_wo1n[:], s["sg"][:], start=False, stop=True)
            # gate l1 matmuls that don't need tode run while ACT does a1
            for c in chunks:
                s = st[c]
                cur = s["hx"][t % 2]
                pzr1[c] = psum.tile([NU, 2 * NCH], F32, tag=f"l1w{c}",
                                    name="pzr1", bufs=1)
                mmul(pzr1[c][:, 0:NCH], s_wz1f[:], cur[:],
                     start=True, stop=False)
                mmul(pzr1[c][:, 0:NCH], s_wz1y[:], s["p"][:],
                     start=False, stop=False)
                mmul(pzr1[c][:, 0:NCH], s_wz1n[:], s["sg"][:],
                     start=False, stop=False)
                mmul(pzr1[c][:, NCH:2 * NCH], s_wr1f[:], cur[:],
                     start=True, stop=False)
                mmul(pzr1[c][:, NCH:2 * NCH], s_wr1y[:], s["p"][:],
                     start=False, stop=False)
                mmul(pzr1[c][:, NCH:2 * NCH], s_wr1n[:], s["sg"][:],
                     start=False, stop=False)
            for c in chunks:
                nc.scalar.activation(st[c]["tode"][0:NU, :], p1[c][:], TANH)
            # ez/er close the gate-l1 accumulations (they gate a2, on-cycle)
            for c in chunks:
                s = st[c]
                mmul(pzr1[c][:, 0:NCH], ez[:], s["tode"][:],
                     start=False, stop=True)
                mmul(pzr1[c][:, NCH:2 * NCH], er[:], s["tode"][:],
                     start=False, stop=True)
            for c in chunks:
                p2[c] = psum.tile([2 * DL, NCH], F32, tag=f"b{c}", name="p2",
                                  bufs=1)
                mmul(p2[c][0:DL, :], s_wo2b[:], st[c]["tode"][:])
            for c in chunks:
                nc.scalar.activation(st[c]["tzr"][0:NU, :], pzr1[c][:], TANH)
            # exact y_ode path (off-cycle): q = p2*dt + m; y_ode = h + q
            for c in chunks:
                s = st[c]
                nc.vector.scalar_tensor_tensor(
                    s["q"][:], p2[c][0:DL, :], s_dts[:, t:t + 1],
                    s["m"][:], op0=MULT, op1=ADD)
            for c in chunks:
                s = st[c]
                nc.vector.tensor_add(s["yode"][:],
                                     s["hx"][t % 2][0:DL, :].bitcast(F32),
                                     s["q"][:])
            # z/r layer 2 stacked in one bank (r 0:64, z 64:128)
            for c in chunks:
                s = st[c]
                pzr2[c] = psum.tile([2 * DL, NCH], F32, tag=f"b{c}",
                                    name="pzr2", bufs=1)
                mmul(pzr2[c][0:DL, :], s_wr2b[:], s["tzr"][:, NCH:2 * NCH])
                mmul(pzr2[c][DL:2 * DL, :], s_wz2b[:], s["tzr"][:, 0:NCH])
            for c in chunks:
                nc.scalar.activation(st[c]["szr"][:], pzr2[c][:], SIG)
            for c in chunks:
                s = st[c]
                # r*y_ode (all base 0)
                nc.vector.tensor_mul(s["rx"][0:DL, :], s["szr"][0:DL, :],
                                     s["yode"][:])
            # z copy to base 0 (off-cycle; z sits in sigmoid rows 64:128)
            for c in chunks:
                s = st[c]
                nc.gpsimd.tensor_copy(s["zc"][:], s["szr"][DL:2 * DL, :])
            for c in chunks:
                ph[c] = psum.tile([NU, NCH], F32, tag=f"l1{c}", name="ph",
                                  bufs=1)
                mmul(ph[c][:], s_wh1f[:], st[c]["rx"][:])
            for c in chunks:
                nc.scalar.activation(st[c]["th"][0:NU, :], ph[c][:], TANH)
            # p' = z*y_ode (off-cycle, Pool)
            for c in chunks:
                s = st[c]
                nc.gpsimd.tensor_mul(s["p"][:], s["zc"][:], s["yode"][:])
            for c in chunks:
                ph2[c] = psum.tile([2 * DL, NCH], F32, tag=f"b{c}",
                                   name="ph2", bufs=1)
                mmul(ph2[c][0:DL, :], s_wh2b[:], st[c]["th"][:])
            for c in chunks:
                s = st[c]
                nc.scalar.activation(s["hx"][(t + 1) % 2][0:DL, :],
                                     ph2[c][0:DL, :], TANH)
            # tail: s' = z*h_new — the only op between a5 and the next step
            for c in chunks:
                s = st[c]
                nc.vector.tensor_mul(s["sg"][:], s["zc"][:],
                                     s["hx"][(t + 1) % 2][0:DL, :].bitcast(F32))

        # y_final = h_99 + p_99 - s_99
        for c in range(CH):
            cs = slice(c * NCH, (c + 1) * NCH)
            s = st[c]
            nc.gpsimd.tensor_sub(s["m"][:], s["p"][:].bitcast(F32),
                                 s["sg"][:].bitcast(F32))
            yfin = singles.tile([DL, NCH], F32R, tag=f"yf{c}", name=f"yf{c}")
            nc.vector.tensor_add(yfin[:],
                                 s["hx"][NSTEP % 2][0:DL, :].bitcast(F32),
                                 s["m"][:])
            nc.sync.dma_start(yout.ap()[:, cs], yfin[:])

    nc.compile()
    return nc


_NC_CACHE = None


def _get_nc():
    global _NC_CACHE
    if _NC_CACHE is None:
        _NC_CACHE = _build()
    return _NC_CACHE


def _prep_core_inputs(data, time_steps, prior, weights):
    """Host-side glue: shard + transpose into the kernel's layouts."""
    import ml_dtypes
    dts = np.concatenate([time_steps[1:2] - time_steps[0:1],
                          time_steps[:-2] - time_steps[1:-1]]).astype(np.float32)
    dts_b = np.ascontiguousarray(
        np.broadcast_to(dts[None, :], (DL, NSTEP))).astype(np.float32)
    (Wo1, bo1, Wo2, bo2, Wz1, bz1, Wz2, bz2,
     Wr1, br1, Wr2, br2, Wh1, bh1, Wh2, bh2) = weights

    def wb(W, b):
        return np.concatenate([W, b[None, :]], axis=0)

    wo2b = wb(Wo2, bo2)                       # [101, 64]
    dt0 = float(dts[0])
    dtr = float(dts[1]) if NSTEP > 1 else dt0
    shared = {
        "wo1f": np.concatenate(
            [Wo1, np.zeros((DD, NU), np.float32), bo1[None, :]], axis=0),
        "wo1y": Wo1, "wo1n": -Wo1,
        "wo2b": wo2b,
        "wz1f": wb(Wz1, bz1), "wz1y": Wz1[:DL], "wz1n": -Wz1[:DL],
        "ezb0": dt0 * (wo2b @ Wz1[:DL]), "ezb": dtr * (wo2b @ Wz1[:DL]),
        "wr1f": wb(Wr1, br1), "wr1y": Wr1[:DL], "wr1n": -Wr1[:DL],
        "erb0": dt0 * (wo2b @ Wr1[:DL]), "erb": dtr * (wo2b @ Wr1[:DL]),
        "wh1f": wb(Wh1, bh1),
        "wh2b": wb(Wh2, bh2),
        "dts": dts_b,
        "zeros": np.zeros((DL, B), np.float32),
        "ones": np.ones((1, B), np.float32),
    }
    shared = {k: np.ascontiguousarray(v, dtype=np.float32)
              for k, v in shared.items()}
    shared["wz2b"] = wb(Wz2, bz2).astype(ml_dtypes.bfloat16)
    shared["wr2b"] = wb(Wr2, br2).astype(ml_dtypes.bfloat16)
    shared["ones16"] = np.ones((1, 2 * B), ml_dtypes.bfloat16)
    in_maps = []
    ones_row = np.ones((1, B), np.float32)
    for i in range(NCORES):
        ts_ = slice(i * B, (i + 1) * B)
        xt = data[ts_, 1:, :].transpose(1, 2, 0)  # [NSTEP, DD, B]
        xs1 = np.concatenate(
            [xt, np.broadcast_to(ones_row, (NSTEP, 1, B))], axis=1)
        xs1 = np.ascontiguousarray(xs1).astype(np.float32)
        pr = np.ascontiguousarray(prior[ts_].T).astype(np.float32)
        in_maps.append({"xs": xs1, "prior": pr, **shared})
    return in_maps


def kernel(data, time_steps, prior,
           Wo1, bo1, Wo2, bo2,
           Wz1, bz1, Wz2, bz2,
           Wr1, br1, Wr2, br2,
           Wh1, bh1, Wh2, bh2):
    data = np.asarray(data, dtype=np.float32)
    time_steps = np.asarray(time_steps, dtype=np.float32)
    prior = np.asarray(prior, dtype=np.float32)
    weights = [np.asarray(w, dtype=np.float32) for w in
               (Wo1, bo1, Wo2, bo2, Wz1, bz1, Wz2, bz2,
                Wr1, br1, Wr2, br2, Wh1, bh1, Wh2, bh2)]
    nc = _get_nc()
    in_maps = _prep_core_inputs(data, time_steps, prior, weights)
    res = run_bass_kernel_spmd(nc, in_maps, core_ids=list(range(NCORES)))
    out = np.empty((N_TRAJ, DL), dtype=np.float32)
    for i in range(NCORES):
        out[i * B:(i + 1) * B] = res.results[i]["yout"].T
    return out


# revision 21
# speedup vs baseline: 1.0397x; 1.0397x over previous
"""ODE-RNN decoder kernel for Trainium2 (8 NeuronCores, data-parallel).

Math per scan step (t = 0..98), per trajectory:
    y_ode = y + (tanh(y @ Wo1 + bo1) @ Wo2 + bo2) * dt_t
    z     = sigmoid(tanh([y_ode;x] @ Wz1 + bz1) @ Wz2 + bz2)
    r     = sigmoid(tanh([y_ode;x] @ Wr1 + br1) @ Wr2 + br2)
    h     = tanh(tanh([r*y_ode;x] @ Wh1 + bh1) @ Wh2 + bh2)
    y     = (1-z)*h + z*y_ode

Layout: feature-major on-chip ([feature, batch]); batch 8192 sharded 8 ways
data-parallel (1024/core, weights replicated), CH=2 chunks of 512 columns.

The step is latency-bound by the recurrence's serial dependency cycle, so
the kernel minimizes that cycle, not engine throughput:

- State is kept SPLIT as y_t = h_t + p_t - s_t with h = gate output,
  p = z*y_ode, s = z*h; y is never materialized in the loop.  p is
  computable mid-step; only s needs the just-produced h, so the post-tanh
  tail is ONE elementwise multiply.
- Moving operands are packed so each GEMM needs few matmuls:
    hp [128, n] = [h; p]     (tanh writes rows 0:64, z*y_ode rows 64:128)
    sx [97, n]  = [s; x; 1]  (tail writes rows 0:64, DMA rows 64:96)
  Layer-1 preacts = W_hp^T@hp + W_sx^T@sx (+ Ez^T@[tode;1] for the gates),
  with W_hp = [W; W], W_sx = [-W; Wx; b] — biases ride the ones row, the
  subtraction rides negated weights, and the ODE update is folded in
  algebraically via Ez = dt*[Wo2;bo2]@Wz1y so the gate path needs only the
  ODE tanh (never y_ode itself).
- The exact y_ode is still formed on the side for the elementwise r*y_ode /
  z*y_ode: w1 = h-s (step head), w2 = w1+p (head), y_ode = dt*p2 + w2 —
  partition bases alternate 0/64 so every tensor-op's INPUTS share a base
  (outputs may differ; verified on hw).
- z/r layer-1 tanh is ONE fused instruction over a 2-bank PSUM tile; z/r
  layer-2 stack on partitions 0:64 (r) / 64:128 (z) of one bank so ONE
  sigmoid covers both (bf16 weights/moving — f32r matmuls cannot write PSUM
  base 64).  z is copied once per step to a base-0 tile (Pool, off-cycle).

Critical cycle: a5(h) -> s-mul -> sx-matmul -> a1(tode) -> ez -> a2 -> l2
-> a3(sigmoid) -> v2(r*yode) -> ph -> a4 -> ph2 -> a5.
"""

import os
import sys

sys.path.insert(0, "/opt/trn_rl_repo")

from contextlib import ExitStack

import numpy as np

import concourse.bass as bass
import concourse.tile as tile
from concourse import bacc, mybir
from concourse.bass_utils import run_bass_kernel_spmd

N_TRAJ, T, DD, DL, NU = 8192, 100, 32, 64, 100
NSTEP = T - 1
NCORES = 8
B = N_TRAJ // NCORES  # 1024 per core
CH = int(os.environ.get("KCH", "2"))    # chunks in flight per core
NCH = B // CH                           # columns per chunk
assert NCH % 8 == 0 and NCH >= 256      # f32r matmul free-dim rules

F32 = mybir.dt.float32
F32R = mybir.dt.float32r
BF16 = mybir.dt.bfloat16
TANH = mybir.ActivationFunctionType.Tanh
SIG = mybir.ActivationFunctionType.Sigmoid
ADD = mybir.AluOpType.add
MULT = mybir.AluOpType.mult


def _build():
    nc = bacc.Bacc("TRN2", target_bir_lowering=False, debug=False)

    def din(name, shape, dt=F32R):
        return nc.dram_tensor(name, list(shape), dt, kind="ExternalInput")

    KS = DL + DD + 1  # 97: [s; x; 1]

    xs = din("xs", [NSTEP, DD, B])         # host: data[:,1:,:] transposed
    prior = din("prior", [DL, B])
    wo1hp = din("wo1hp", [2 * DL, NU])     # [Wo1; Wo1]
    wo1sx = din("wo1sx", [KS, NU])         # [-Wo1; 0; bo1]
    wo2b = din("wo2b", [NU + 1, DL])       # [Wo2; bo2]
    wz1hp = din("wz1hp", [2 * DL, NU])     # [Wz1y; Wz1y]
    wz1sx = din("wz1sx", [KS, NU])         # [-Wz1y; Wz1x; bz1]
    ezb0 = din("ezb0", [NU + 1, NU])       # dt0 * [Wo2;bo2] @ Wz1y
    ezb = din("ezb", [NU + 1, NU])         # dtr * [Wo2;bo2] @ Wz1y
    wr1hp = din("wr1hp", [2 * DL, NU])
    wr1sx = din("wr1sx", [KS, NU])
    erb0 = din("erb0", [NU + 1, NU])
    erb = din("erb", [NU + 1, NU])
    wh1f = din("wh1f", [KS, NU])           # [Wh1y; Wh1x; bh1]
    wz2b = din("wz2b", [NU + 1, DL], BF16)  # [Wz2; bz2]
    wr2b = din("wr2b", [NU + 1, DL], BF16)  # [Wr2; br2]
    wh2b = din("wh2b", [NU + 1, DL])        # [Wh2; bh2]
    dts = din("dts", [DL, NSTEP], F32)      # exact per-step dt (y_ode path)
    zeros = din("zeros", [DL, B])           # p_0 = s_0 = 0
    ones = din("ones", [1, B])              # f32r ones rows
    ones16 = din("ones16", [1, 2 * B], BF16)
    yout = nc.dram_tensor("yout", [DL, B], F32R, kind="ExternalOutput")

    mmul = nc.tensor.matmul

    with tile.TileContext(nc) as tc, ExitStack() as ctx:
        singles = ctx.enter_context(tc.tile_pool(name="singles", bufs=1))
        psum = ctx.enter_context(tc.tile_pool(name="psum", bufs=1, space="PSUM"))

        def load(dr, shape, dt=F32R):
            t_ = singles.tile(shape, dt, tag=dr.name, name="s_" + dr.name)
            nc.sync.dma_start(t_[:], dr.ap())
            return t_

        s_wo1hp = load(wo1hp, [2 * DL, NU])
        s_wo1sx = load(wo1sx, [KS, NU])
        s_wo2b = load(wo2b, [NU + 1, DL])
        s_wz1hp = load(wz1hp, [2 * DL, NU])
        s_wz1sx = load(wz1sx, [KS, NU])
        s_ezb0 = load(ezb0, [NU + 1, NU])
        s_ezb = load(ezb, [NU + 1, NU])
        s_wr1hp = load(wr1hp, [2 * DL, NU])
        s_wr1sx = load(wr1sx, [KS, NU])
        s_erb0 = load(erb0, [NU + 1, NU])
        s_erb = load(erb, [NU + 1, NU])
        s_wh1f = load(wh1f, [KS, NU])
        s_wz2b = load(wz2b, [NU + 1, DL], BF16)
        s_wr2b = load(wr2b, [NU + 1, DL], BF16)
        s_wh2b = load(wh2b, [NU + 1, DL])
        s_dts = load(dts, [DL, NSTEP], F32)

        # per-chunk persistent state tiles
        st = {}
        for c in range(CH):
            cs = slice(c * NCH, (c + 1) * NCH)
            # double-buffered state: step t reads hp[t%2]/sx[t%2]; a5 / p-mul
            # / s-mul / x-DMA of step t write into buffer (t+1)%2
            hp, sx = [], []
            for j in range(2):
                hpj = singles.tile([2 * DL, NCH], F32R, tag=f"hp{c}_{j}",
                                   name=f"hp{c}_{j}")
                sxj = singles.tile([KS, NCH], F32R, tag=f"sx{c}_{j}",
                                   name=f"sx{c}_{j}")
                nc.sync.dma_start(sxj[DL + DD:KS, :], ones.ap()[:, 0:NCH])
                hp.append(hpj)
                sx.append(sxj)
            nc.sync.dma_start(hp[0][0:DL, :], prior.ap()[:, cs])
            nc.sync.dma_start(hp[0][DL:2 * DL, :], zeros.ap()[:, cs])
            nc.sync.dma_start(sx[0][0:DL, :], zeros.ap()[:, cs])
            nc.sync.dma_start(sx[0][DL:DL + DD, :], xs.ap()[0, :, cs])
            rx = singles.tile([KS, NCH], F32R, tag=f"rx{c}", name=f"rx{c}")
            nc.sync.dma_start(rx[DL + DD:KS, :], ones.ap()[:, 0:NCH])
            tode = singles.tile([NU + 1, NCH], F32R, tag=f"to{c}", name=f"to{c}")
            nc.sync.dma_start(tode[NU:NU + 1, :], ones.ap()[:, 0:NCH])
            tzr = singles.tile([NU + 1, 2 * NCH], BF16, tag=f"tzr{c}",
                               name=f"tzr{c}")
            nc.sync.dma_start(tzr[NU:NU + 1, :], ones16.ap()[:, 0:2 * NCH])
            th = singles.tile([NU + 1, NCH], F32R, tag=f"th{c}", name=f"th{c}")
            nc.sync.dma_start(th[NU:NU + 1, :], ones.ap()[:, 0:NCH])
            st[c] = dict(
                hp=hp, sx=sx, rx=rx, tode=tode, tzr=tzr, th=th,
                w=singles.tile([2 * DL, NCH], F32, tag=f"w{c}", name=f"w{c}"),
                w2=singles.tile([DL, NCH], F32, tag=f"w2{c}", name=f"w2{c}"),
                yode=singles.tile([DL, NCH], F32, tag=f"yo{c}", name=f"yo{c}"),
                zc=singles.tile([DL, NCH], F32, tag=f"zc{c}", name=f"zc{c}"),
                szr=singles.tile([2 * DL, NCH], F32, tag=f"szr{c}",
                                 name=f"szr{c}"),
            )

        # Stage-major emission across chunks: each engine's in-order queue
        # alternates chunks per stage (no head-of-line blocking).
        chunks = list(range(CH))
        for t in range(NSTEP):
            p1 = {}; p2 = {}; pzr1 = {}; pzr2 = {}; ph = {}; ph2 = {}
            ez = s_ezb0 if t == 0 else s_ezb
            er = s_erb0 if t == 0 else s_erb
            for c in chunks:
                s = st[c]
                cs = slice(c * NCH, (c + 1) * NCH)
                if t + 1 < NSTEP:
                    nc.sync.dma_start(s["sx"][(t + 1) % 2][DL:DL + DD, :],
                                      xs.ap()[t + 1, :, cs])
                nc.sync.dma_start(s["rx"][DL:DL + DD, :], xs.ap()[t, :, cs])
            # step head (all operands from the previous step):
            # w1 = h - s at base 64, w2 = w1 + p at base 0
            for c in chunks:
                s = st[c]
                cur_hp, cur_sx = s["hp"][t % 2], s["sx"][t % 2]
                nc.gpsimd.tensor_sub(s["w"][DL:2 * DL, :],
                                     cur_hp[0:DL, :].bitcast(F32),
                                     cur_sx[0:DL, :].bitcast(F32))
            for c in chunks:
                s = st[c]
                nc.gpsimd.tensor_add(s["w2"][:], s["w"][DL:2 * DL, :],
                                     s["hp"][t % 2][DL:2 * DL, :].bitcast(F32))
            # ODE layer 1: p1 = Wo1^T(h+p-s) + bo1
            for c in chunks:
                s = st[c]
                p1[c] = psum.tile([NU, NCH], F32, tag=f"l1{c}", name="p1",
                                  bufs=1)
                mmul(p1[c][:], s_wo1hp[:], s["hp"][t % 2][:],
                     start=True, stop=False)
                mmul(p1[c][:], s_wo1sx[:], s["sx"][t % 2][:],
                     start=False, stop=True)
            # gate layer-1 hp/sx parts run while ACT does a1
            for c in chunks:
                s = st[c]
                cur_hp, cur_sx = s["hp"][t % 2], s["sx"][t % 2]
                pzr1[c] = psum.tile([NU, 2 * NCH], F32, tag=f"l1w{c}",
                                    name="pzr1", bufs=1)
                mmul(pzr1[c][:, 0:NCH], s_wz1hp[:], cur_hp[:],
                     start=True, stop=False)
                mmul(pzr1[c][:, 0:NCH], s_wz1sx[:], cur_sx[:],
                     start=False, stop=False)
                mmul(pzr1[c][:, NCH:2 * NCH], s_wr1hp[:], cur_hp[:],
                     start=True, stop=False)
                mmul(pzr1[c][:, NCH:2 * NCH], s_wr1sx[:], cur_sx[:],
                     start=False, stop=False)
            for c in chunks:
                nc.scalar.activation(st[c]["tode"][0:NU, :], p1[c][:], TANH)
            # ez/er close the gate-l1 accumulations (they gate a2, on-cycle)
            for c in chunks:
                s = st[c]
                mmul(pzr1[c][:, 0:NCH], ez[:], s["tode"][:],
                     start=False, stop=True)
                mmul(pzr1[c][:, NCH:2 * NCH], er[:], s["tode"][:],
                     start=False, stop=True)
            for c in chunks:
                p2[c] = psum.tile([2 * DL, NCH], F32, tag=f"b{c}", name="p2",
                                  bufs=1)
                mmul(p2[c][0:DL, :], s_wo2b[:], st[c]["tode"][:])
            for c in chunks:
                nc.scalar.activation(st[c]["tzr"][0:NU, :], pzr1[c][:], TANH)
            # exact y_ode = dt*p2 + (h+p-s)  (off-cycle)
            for c in chunks:
                s = st[c]
                nc.vector.scalar_tensor_tensor(
                    s["yode"][:], p2[c][0:DL, :], s_dts[:, t:t + 1],
                    s["w2"][:], op0=MULT, op1=ADD)
            # z/r layer 2 stacked in one bank (r 0:64, z 64:128)
            for c in chunks:
                s = st[c]
                pzr2[c] = psum.tile([2 * DL, NCH], F32, tag=f"b{c}",
                                    name="pzr2", bufs=1)
                mmul(pzr2[c][0:DL, :], s_wr2b[:], s["tzr"][:, NCH:2 * NCH])
                mmul(pzr2[c][DL:2 * DL, :], s_wz2b[:], s["tzr"][:, 0:NCH])
            for c in chunks:
                nc.scalar.activation(st[c]["szr"][:], pzr2[c][:], SIG)
            for c in chunks:
                s = st[c]
                # r*y_ode (all base 0)
                nc.vector.tensor_mul(s["rx"][0:DL, :], s["szr"][0:DL, :],
                                     s["yode"][:])
            # z copy to base 0 (off-cycle; z sits in sigmoid rows 64:128)
            for c in chunks:
                s = st[c]
                nc.gpsimd.tensor_copy(s["zc"][:], s["szr"][DL:2 * DL, :])
            for c in chunks:
                ph[c] = psum.tile([NU, NCH], F32, tag=f"l1{c}", name="ph",
                                  bufs=1)
                mmul(ph[c][:], s_wh1f[:], st[c]["rx"][:])
            for c in chunks:
                nc.scalar.activation(st[c]["th"][0:NU, :], ph[c][:], TANH)
            # p' = z*y_ode into hp' rows 64:128 (off-cycle, Pool)
            for c in chunks:
                s = st[c]
                nc.gpsimd.tensor_mul(s["hp"][(t + 1) % 2][DL:2 * DL, :],
                                     s["zc"][:], s["yode"][:])
            for c in chunks:
                ph2[c] = psum.tile([2 * DL, NCH], F32, tag=f"b{c}",
                                   name="ph2", bufs=1)
                mmul(ph2[c][0:DL, :], s_wh2b[:], st[c]["th"][:])
            for c in chunks:
                s = st[c]
                nc.scalar.activation(s["hp"][(t + 1) % 2][0:DL, :],
                                     ph2[c][0:DL, :], TANH)
            # tail: s' = z*h_new — the only op between a5 and the next step
            for c in chunks:
                s = st[c]
                nc.vector.tensor_mul(s["sx"][(t + 1) % 2][0:DL, :], s["zc"][:],
                                     s["hp"][(t + 1) % 2][0:DL, :].bitcast(F32))

        # y_final = h_99 + p_99 - s_99
        for c in range(CH):
            cs = slice(c * NCH, (c + 1) * NCH)
            s = st[c]
            jf = NSTEP % 2
            nc.gpsimd.tensor_sub(s["w"][DL:2 * DL, :],
                                 s["hp"][jf][0:DL, :].bitcast(F32),
                                 s["sx"][jf][0:DL, :].bitcast(F32))
            yfin = singles.tile([DL, NCH], F32R, tag=f"yf{c}", name=f"yf{c}")
            nc.vector.tensor_add(yfin[:], s["w"][DL:2 * DL, :],
                                 s["hp"][jf][DL:2 * DL, :].bitcast(F32))
            nc.sync.dma_start(yout.ap()[:, cs], yfin[:])

    nc.compile()
    return nc


_NC_CACHE = None


def _get_nc():
    global _NC_CACHE
    if _NC_CACHE is None:
        _NC_CACHE = _build()
    return _NC_CACHE


def _prep_core_inputs(data, time_steps, prior, weights):
    """Host-side glue: shard + transpose into the kernel's layouts."""
    import ml_dtypes
    dts = np.concatenate([time_steps[1:2] - time_steps[0:1],
                          time_steps[:-2] - time_steps[1:-1]]).astype(np.float32)
    dts_b = np.ascontiguousarray(
        np.broadcast_to(dts[None, :], (DL, NSTEP))).astype(np.float32)
    (Wo1, bo1, Wo2, bo2, Wz1, bz1, Wz2, bz2,
     Wr1, br1, Wr2, br2, Wh1, bh1, Wh2, bh2) = weights

    def wb(W, b):
        return np.concatenate([W, b[None, :]], axis=0)

    def hpw(Wy):
        return np.concatenate([Wy, Wy], axis=0)           # [128, 100]

    def sxw(Wy, Wx, b):
        return np.concatenate([-Wy, Wx, b[None, :]], axis=0)  # [97, 100]

    wo2b = wb(Wo2, bo2)                       # [101, 64]
    dt0 = float(dts[0])
    dtr = float(dts[1]) if NSTEP > 1 else dt0
    z32 = np.zeros((DD, NU), np.float32)
    shared = {
        "wo1hp": hpw(Wo1), "wo1sx": sxw(Wo1, z32, bo1),
        "wo2b": wo2b,
        "wz1hp": hpw(Wz1[:DL]), "wz1sx": sxw(Wz1[:DL], Wz1[DL:], bz1),
        "ezb0": dt0 * (wo2b @ Wz1[:DL]), "ezb": dtr * (wo2b @ Wz1[:DL]),
        "wr1hp": hpw(Wr1[:DL]), "wr1sx": sxw(Wr1[:DL], Wr1[DL:], br1),
        "erb0": dt0 * (wo2b @ Wr1[:DL]), "erb": dtr * (wo2b @ Wr1[:DL]),
        "wh1f": wb(Wh1, bh1),
        "wh2b": wb(Wh2, bh2),
        "dts": dts_b,
        "zeros": np.zeros((DL, B), np.float32),
        "ones": np.ones((1, B), np.float32),
    }
    shared = {k: np.ascontiguousarray(v, dtype=np.float32)
              for k, v in shared.items()}
    shared["wz2b"] = wb(Wz2, bz2).astype(ml_dtypes.bfloat16)
    shared["wr2b"] = wb(Wr2, br2).astype(ml_dtypes.bfloat16)
    shared["ones16"] = np.ones((1, 2 * B), ml_dtypes.bfloat16)
    in_maps = []
    for i in range(NCORES):
        ts_ = slice(i * B, (i + 1) * B)
        xt = np.ascontiguousarray(
            data[ts_, 1:, :].transpose(1, 2, 0)).astype(np.float32)
        pr = np.ascontiguousarray(prior[ts_].T).astype(np.float32)
        in_maps.append({"xs": xt, "prior": pr, **shared})
    return in_maps


def kernel(data, time_steps, prior,
           Wo1, bo1, Wo2, bo2,
           Wz1, bz1, Wz2, bz2,
           Wr1, br1, Wr2, br2,
           Wh1, bh1, Wh2, bh2):
    data = np.asarray(data, dtype=np.float32)
    time_steps = np.asarray(time_steps, dtype=np.float32)
    prior = np.asarray(prior, dtype=np.float32)
    weights = [np.asarray(w, dtype=np.float32) for w in
               (Wo1, bo1, Wo2, bo2, Wz1, bz1, Wz2, bz2,
                Wr1, br1, Wr2, br2, Wh1, bh1, Wh2, bh2)]
    nc = _get_nc()
    in_maps = _prep_core_inputs(data, time_steps, prior, weights)
    res = run_bass_kernel_spmd(nc, in_maps, core_ids=list(range(NCORES)))
    out = np.empty((N_TRAJ, DL), dtype=np.float32)
    for i in range(NCORES):
        out[i * B:(i + 1) * B] = res.results[i]["yout"].T
    return out
